# revision 8
# baseline (speedup 1.0000x reference)
"""Any4 quantized linear (LUT dequant + GEMM + bias) on 8 Trainium2 cores.

Strategy: column-parallel over out_features O=4096 -> 512 per core.
Per core:
  prologue: dequantize W[512, 4096] from int4 codes via 16 mask-accumulate
    passes on the Vector engine (per-partition LUT scalars), then per-group
    (group=128) affine scale/zero; transpose W to [I, O_shard] via the PE
    (identity matmul) into SBUF as bf16 (resident, 32 k-tiles of [128, 512]).
  main loop over 16 m-blocks of 512 rows of x:
    - SWDGE cast-DMA x f32 -> bf16 tiles [128, 4096]
    - per k-tile: 4 PE transposes build xT [i=128, m=512] in PSUM,
      ACT copies to SBUF; 4 matmuls accumulate y[m=128, o=512] in PSUM
      (initialized with a K=1 ones x bias matmul)
    - ACT copies y PSUM -> SBUF, DMA out f32
Host: concatenate the 8 [8192, 512] shards along axis 1.

Self-contained: hardcodes shapes M=8192, I=4096, O=4096, G=128, n_cores=8.
"""
import sys

sys.path.insert(0, "/opt/trn_rl_repo")

import numpy as np

import concourse.bass as bass
import concourse.mybir as mybir
import bass_rust
from concourse import tile
from concourse.bass_utils import run_bass_kernel_spmd

M, I, O, G = 8192, 4096, 4096, 128
NCORES = 8
OSH = O // NCORES          # 512 out features per core
P = 128                    # partitions
KT = I // P                # 32 k-tiles
MB = 512                   # m-block rows
NMB = M // MB              # 16 m-blocks
MSUB = MB // P             # 4 m-sub tiles per block
BF = mybir.dt.bfloat16
F32 = mybir.dt.float32


def _split_waits(nc, budget=1, noop_budget=1):
    """walrus in this toolchain rejects instructions with >1 embedded sem
    wait; move excess waits onto same-engine NoOp carriers placed directly
    before the instruction."""
    ctr = 0
    for fn in nc.m.functions:
        for bb in fn.blocks:
            lst = bb.instructions
            i = 0
            while i < len(lst):
                inst = lst[i]
                si = inst.sync_info
                if si is None:
                    i += 1
                    continue
                waits = list(si.on_wait or [])
                if len(waits) <= budget:
                    i += 1
                    continue
                inst.sync_info = bass_rust.SyncInfo(
                    on_wait=waits[:budget], on_update=list(si.on_update or []))
                excess = waits[budget:]
                cars = []
                for j in range(0, len(excess), noop_budget):
                    ctr += 1
                    n = mybir.InstNoOp(name=f"waitc-{ctr}", ins=[], outs=[])
                    n.engine = inst.engine
                    n.sync_info = bass_rust.SyncInfo(
                        on_wait=excess[j:j + noop_budget], on_update=[])
                    cars.append(n)
                for j, c in enumerate(cars):
                    lst.insert(i + j, c)
                i += 1 + len(cars)
    return ctr


def build(nmb=NMB, skip_dequant=False, skip_main=False, skip_wt=False, skip_deq_ops=False):
    nc = bass.Bass()
    x_d = nc.dram_tensor("x", [M, I], F32, kind="ExternalInput")
    codes_d = nc.dram_tensor("codes", [OSH, I], mybir.dt.int32, kind="ExternalInput")
    lut_d = nc.dram_tensor("lut", [OSH, 16], F32, kind="ExternalInput")
    scale_d = nc.dram_tensor("scale", [OSH, I // G], F32, kind="ExternalInput")
    zero_d = nc.dram_tensor("zero", [OSH, I // G], F32, kind="ExternalInput")
    bias_d = nc.dram_tensor("bias", [1, OSH], F32, kind="ExternalInput")
    ident_d = nc.dram_tensor("ident", [P, P], BF, kind="ExternalInput")
    y_d = nc.dram_tensor("y", [M, OSH], F32, kind="ExternalOutput")

    NG = I // G  # 32 groups
    OT = OSH // P  # 4 o-tiles

    with tile.TileContext(nc) as tc:
        with (
            tc.tile_pool(name="const", bufs=1) as cpool,
            tc.tile_pool(name="deq", bufs=2) as dpool,
            tc.tile_pool(name="xp", bufs=2) as xpool,
            tc.tile_pool(name="xtp", bufs=3) as xtpool,
            tc.tile_pool(name="yp", bufs=2) as ypool,
            tc.tile_pool(name="psx", bufs=2, space="PSUM") as psx,
            tc.tile_pool(name="psy", bufs=1, space="PSUM") as psy,
        ):
            ident = cpool.tile([P, P], BF, tag="ident")
            nc.sync.dma_start(ident[:], ident_d[:])
            ones = cpool.tile([P, P], BF, tag="ones")
            nc.vector.memset(ones[:], 1.0)
            bias_bf = cpool.tile([P, OSH], BF, tag="bias")
            nc.gpsimd.dma_start(bias_bf[0:1, :], bias_d[:])

            # resident transposed weights: wt[k] = W^T[k*128:(k+1)*128, :OSH] bf16
            wt = [cpool.tile([P, OSH], BF, tag=f"wt{k}", name=f"wt{k}")
                  for k in range(KT)]

            # ---------------- prologue: dequant + transpose W ----------------
            for ot in range(OT if not skip_dequant else 0):
                osl = slice(ot * P, (ot + 1) * P)
                codes_i = dpool.tile([P, I], mybir.dt.int32, tag="codes_i")
                nc.sync.dma_start(codes_i[:], codes_d[osl, :])
                codes_bf = dpool.tile([P, I], BF, tag="codes_bf")
                nc.vector.tensor_copy(codes_bf[:], codes_i[:])
                lut_sb = dpool.tile([P, 16], F32, tag="lut")
                nc.sync.dma_start(lut_sb[:], lut_d[osl, :])
                scale_sb = dpool.tile([P, NG], F32, tag="scale")
                nc.sync.dma_start(scale_sb[:], scale_d[osl, :])
                zero_sb = dpool.tile([P, NG], F32, tag="zero")
                nc.sync.dma_start(zero_sb[:], zero_d[osl, :])

                # wlut[o, i] = lut[o, codes[o, i]]
                wl = dpool.tile([P, I], BF, tag="wl")
                if skip_deq_ops:
                    nc.vector.memset(wl[:], 0.01)
                if not skip_deq_ops:
                    nc.vector.tensor_scalar(
                        wl[:], codes_bf[:], 0.0, lut_sb[:, 0:1],
                        mybir.AluOpType.is_equal, mybir.AluOpType.mult)
                for v in range(1, 16 if not skip_deq_ops else 1):
                    ms = dpool.tile([P, I], BF, tag="ms")
                    nc.vector.tensor_scalar(
                        ms[:], codes_bf[:], float(v), lut_sb[:, v:v + 1],
                        mybir.AluOpType.is_equal, mybir.AluOpType.mult)
                    nc.vector.tensor_tensor(
                        wl[:], wl[:], ms[:], mybir.AluOpType.add)
                # per-group affine: w = wlut * scale[g] + zero[g]
                wb = dpool.tile([P, I], BF, tag="wb")
                if skip_deq_ops:
                    nc.vector.tensor_copy(wb[:], wl[:])
                for g in range(NG if not skip_deq_ops else 0):
                    gs = slice(g * G, (g + 1) * G)
                    nc.vector.tensor_scalar(
                        wb[:, gs], wl[:, gs], scale_sb[:, g:g + 1],
                        zero_sb[:, g:g + 1],
                        mybir.AluOpType.mult, mybir.AluOpType.add)
                # transpose into resident wt tiles
                if skip_wt:
                    for k in range(KT):
                        nc.vector.tensor_copy(
                            wt[k][:, ot * P:(ot + 1) * P],
                            wb[:, k * P:(k + 1) * P])
                else:
                    for k in range(KT):
                        tp = psx.tile([P, MB], BF, tag="xtp", name="wtp")
                        nc.tensor.transpose(
                            tp[:, 0:P], wb[:, k * P:(k + 1) * P], ident[:])
                        nc.scalar.copy(wt[k][:, ot * P:(ot + 1) * P], tp[:, 0:P])

            if skip_dequant:
                for k in range(KT):
                    nc.vector.memset(wt[k][:], 0.01)
            # ---------------- main loop ----------------
            for mb in range(nmb if not skip_main else 0):
                xs = []
                for a in range(MSUB):
                    xt = xpool.tile([P, I], BF, tag=f"x{a}")
                    r0 = mb * MB + a * P
                    nc.gpsimd.dma_start(xt[:], x_d[r0:r0 + P, :])  # f32->bf16
                    xs.append(xt)
                yps = []
                for a in range(MSUB):
                    yp = psy.tile([P, OSH], F32, tag=f"y{a}")
                    nc.tensor.matmul(yp[:], ones[0:1, :], bias_bf[0:1, :],
                                     start=True, stop=False)
                    yps.append(yp)
                for k in range(KT):
                    ks = slice(k * P, (k + 1) * P)
                    xtp_ps = psx.tile([P, MB], BF, tag="xtp")
                    for a in range(MSUB):
                        nc.tensor.transpose(
                            xtp_ps[:, a * P:(a + 1) * P], xs[a][:, ks], ident[:])
                    xt_sb = xtpool.tile([P, MB], BF, tag="xt")
                    nc.scalar.copy(xt_sb[:], xtp_ps[:])
                    last = (k == KT - 1)
                    for a in range(MSUB):
                        nc.tensor.matmul(
                            yps[a][:], xt_sb[:, a * P:(a + 1) * P], wt[k][:],
                            start=False, stop=last)
                for a in range(MSUB):
                    ysb = ypool.tile([P, OSH], F32, tag="ysb")
                    nc.scalar.copy(ysb[:], yps[a][:])
                    r0 = mb * MB + a * P
                    nc.sync.dma_start(y_d[r0:r0 + P, :], ysb[:])

    _split_waits(nc)
    return nc


def _install_ntff_shim():
    """This image's antenv lacks axon_hooks, so run_bass_kernel_spmd's
    trace=True path can't find the NTFF profile hook. Recreate it: a tiny
    antenv.axon_hooks module plus the ctypes hook into libaxon_pjrt.so
    (same mechanism as trn_agent_boot)."""
    import types, contextlib, ctypes, os as _os
    if "antenv.axon_hooks" in sys.modules:
        return
    mod = types.ModuleType("antenv.axon_hooks")
    holder = {}
    mod.set_axon_ntff_profile_hook = lambda h: holder.__setitem__("h", h)
    mod.get_axon_ntff_profile_hook = lambda: holder.get("h")
    sys.modules["antenv.axon_hooks"] = mod
    try:
        import antenv
        antenv.axon_hooks = mod
    except ImportError:
        pass
    so_path = "/opt/axon/libaxon_pjrt.so"
    if not _os.path.exists(so_path):
        return
    lib = ctypes.CDLL(so_path)
    if not hasattr(lib, "axon_start_nrt_profile"):
        return
    lib.axon_start_nrt_profile.argtypes = [
        ctypes.POINTER(ctypes.c_int64), ctypes.c_size_t]
    lib.axon_start_nrt_profile.restype = ctypes.c_int64
    lib.axon_stop_nrt_profile.argtypes = [ctypes.c_char_p]
    lib.axon_stop_nrt_profile.restype = ctypes.c_int64

    @contextlib.contextmanager
    def _hook(output_dir, device_ids):
        import jax
        jax.devices()
        if device_ids:
            ids = (ctypes.c_int64 * len(device_ids))(*device_ids)
            rc = lib.axon_start_nrt_profile(ids, len(device_ids))
        else:
            rc = lib.axon_start_nrt_profile(None, 0)
        if rc != 0:
            raise RuntimeError(f"axon_start_nrt_profile rc={rc}")
        try:
            yield
        finally:
            n = lib.axon_stop_nrt_profile(str(output_dir).encode())
            print(f"ntff profile: {n} file(s) written to {output_dir}")

    mod.set_axon_ntff_profile_hook(_hook)


_NC_CACHE = None
_BUILD_KW = {}


def _get_nc():
    global _NC_CACHE
    if _NC_CACHE is None:
        _NC_CACHE = build(**_BUILD_KW)
    return _NC_CACHE


def _make_in_maps(input, weight, lut, scales_and_zeros, bias):
    x = np.ascontiguousarray(np.asarray(input, dtype=np.float32))
    codes = np.asarray(weight, dtype=np.int32)
    lut = np.asarray(lut, dtype=np.float32)
    sz = np.asarray(scales_and_zeros, dtype=np.float32)
    bias = np.asarray(bias, dtype=np.float32)
    scaleT = np.ascontiguousarray(sz[..., 0].T)  # [O, I//G]
    zeroT = np.ascontiguousarray(sz[..., 1].T)
    import ml_dtypes
    ident_bf = np.eye(P, dtype=ml_dtypes.bfloat16)
    in_maps = []
    for c in range(NCORES):
        osl = slice(c * OSH, (c + 1) * OSH)
        in_maps.append({
            "x": x,
            "codes": np.ascontiguousarray(codes[osl]),
            "lut": np.ascontiguousarray(lut[osl]),
            "scale": np.ascontiguousarray(scaleT[osl]),
            "zero": np.ascontiguousarray(zeroT[osl]),
            "bias": np.ascontiguousarray(bias[osl]).reshape(1, OSH),
            "ident": ident_bf,
        })
    return in_maps


def run(input, weight, lut, scales_and_zeros, bias, trace=False, tmpdir=None):
    if trace:
        _install_ntff_shim()
        import concourse.bass_utils as _bu
        _bu.upload_artifacts = lambda d: d  # zero-egress container
    nc = _get_nc()
    in_maps = _make_in_maps(input, weight, lut, scales_and_zeros, bias)
    res = run_bass_kernel_spmd(
        nc, in_maps, list(range(NCORES)), trace=trace, tmpdir=tmpdir)
    shards = [res.results[c]["y"] for c in range(NCORES)]
    y = np.concatenate(shards, axis=1)
    return y, res


def kernel(input, weight, lut, scales_and_zeros, bias):
    orig_shape = np.asarray(input).shape
    y, _ = run(input, weight, lut, scales_and_zeros, bias, trace=False)
    return y.reshape(*orig_shape[:-1], O)
